# revision 3
# baseline (speedup 1.0000x reference)
"""Trainium2 Bass kernel for nn_Attention: full attention layer
(QKV proj + per-head RMSNorm on q,k + softmax attention + out proj),
data-parallel over batch across 8 NeuronCores (2 batch elems per core).

Per-core dataflow (bf16 compute, f32 PSUM/stats):
  A. x [tok, hid] tiles -> cast-DMA to bf16 -> PE transpose -> xT [hid, tok].
     First weight chunk + first x tile are prefetched in column slices so
     the PE starts within ~3us of kernel entry.
  B. QKV proj in layout [tok, outdim]: lhsT = xT tiles (stationary),
     rhs = w_qkv.T chunks (head-aligned widths 432/432/288); bias added
     during the DVE PSUM evacuation. q,k stored bf16 per token-tile; v
     lands in v_aug tiles [128, 16*97] where each head owns 97 cols:
     72 v | 24 zero | 1 one.
  C. RMSNorm: ACT Square + DVE reduce -> sumsq (one stats tile per batch);
     one ACT Sqrt + one DVE reciprocal per batch; applied in-place via
     broadcast-AP multiply. gamma_q*gamma_k folds into kT per-head scale.
  D. Attention per head, scores TRANSPOSED: sT[j,i] = kT_j.T @ qT so the
     softmax axis is on partitions; exp on ScalarE over [128, 1024] psum
     pairs (no max subtraction: |logit| <= sqrt(72) after RMSNorm);
     PV lhsT = v_aug head slice [128, 97] -> accumulator row 96 is the
     softmax denominator. Normalize: DVE reciprocal of the denominator row
     [1,S], gpsimd partition_broadcast to [72,S], gpsimd multiply into a
     staging tile, then SBUF->SBUF DMA packs head h's 72 rows at flat row
     72h of 9 dense attn k-tiles (no padding). Scores are software-
     pipelined one jt ahead; next head's PE transposes + next batch's
     x-phase interleave into the loop to keep the PE warm.
  E. Out proj over the 9 packed k-tiles of w_proj.T (bf16) + f32 bias.
"""
import sys
import numpy as np

sys.path.insert(0, "/opt/trn_rl_repo")

import concourse.bass as bass  # noqa: E402,F401
import concourse.tile as tile  # noqa: E402
import concourse.mybir as mybir  # noqa: E402
from concourse import bacc  # noqa: E402
from concourse.bass_utils import run_bass_kernel_spmd  # noqa: E402
from concourse.masks import make_identity  # noqa: E402
import ml_dtypes  # noqa: E402

F32 = mybir.dt.float32
F32R = mybir.dt.float32r
BF16 = mybir.dt.bfloat16
AF = mybir.ActivationFunctionType
MUL = mybir.AluOpType.mult
ADD = mybir.AluOpType.add

B, S, H = 16, 1024, 1152
NH, HD = 16, 72
B_LOCAL = 2
N_CORES = 8
TT = S // 128             # 8 token tiles per batch
CHUNKS = [(0, 432), (432, 432), (864, 288)]   # head-aligned proj chunks
KT_O = H // 128           # 9 packed K-tiles for out proj
NP = 384                  # out-proj N chunk
SCALE = 1.0 / float(np.sqrt(HD))
EPS = float(np.finfo(np.float32).eps)


def build_nc(n_batch=B_LOCAL):
    nc = bacc.Bacc("TRN2", target_bir_lowering=False, debug=False,
                   num_devices=N_CORES)
    x_d = nc.dram_tensor("x", [n_batch, S, H], F32, kind="ExternalInput").ap()
    wqkv_d = nc.dram_tensor("wqkvt", [H, 3 * H], BF16, kind="ExternalInput").ap()
    bias_d = nc.dram_tensor("biasb", [128, 3 * H], BF16, kind="ExternalInput").ap()
    gqk_d = nc.dram_tensor("gqk", [HD, 1], F32, kind="ExternalInput").ap()
    wp_d = nc.dram_tensor("wprojt", [H, H], BF16, kind="ExternalInput").ap()
    bp_d = nc.dram_tensor("bprojb", [128, H], F32, kind="ExternalInput").ap()
    out_d = nc.dram_tensor("out", [n_batch, S, H], F32, kind="ExternalOutput").ap()

    with tile.TileContext(nc) as tc:
        _build(nc, tc, n_batch, x_d, wqkv_d, bias_d, gqk_d, wp_d, bp_d, out_d)
    nc.compile()
    return nc


def _build(nc, tc, n_batch, x_d, wqkv_d, bias_d, gqk_d, wp_d, bp_d, out_d):
    import contextlib
    ctx = contextlib.ExitStack()
    with ctx:
        sbc = ctx.enter_context(tc.tile_pool(name="const", bufs=1))
        sbx = ctx.enter_context(tc.tile_pool(name="sbx", bufs=1))
        sbqk = ctx.enter_context(tc.tile_pool(name="sbqk", bufs=1))
        sbv = ctx.enter_context(tc.tile_pool(name="sbv", bufs=1))
        sba = ctx.enter_context(tc.tile_pool(name="sba", bufs=1))
        sbw = ctx.enter_context(tc.tile_pool(name="sbw", bufs=2))
        sbt = ctx.enter_context(tc.tile_pool(name="sbt", bufs=2))
        sbqt = ctx.enter_context(tc.tile_pool(name="sbqt", bufs=2))
        sbs = ctx.enter_context(tc.tile_pool(name="sbs", bufs=1))
        sbr = ctx.enter_context(tc.tile_pool(name="sbr", bufs=3))
        sbe = ctx.enter_context(tc.tile_pool(name="sbe", bufs=3))
        ps_s = ctx.enter_context(tc.tile_pool(name="pss", bufs=2, space="PSUM"))
        ps_sc = ctx.enter_context(tc.tile_pool(name="pssc", bufs=2, space="PSUM"))
        ps_pv = ctx.enter_context(tc.tile_pool(name="pspv", bufs=1, space="PSUM"))

        # constants
        id32 = sbc.tile([128, 128], F32)
        make_identity(nc, id32[:])
        id16 = sbc.tile([128, 128], BF16)
        nc.vector.tensor_copy(id16[:], id32[:])
        bias_b = sbc.tile([128, 3 * H], BF16)
        nc.sync.dma_start(bias_b[:], bias_d[:])
        zo = sbc.tile([128, 25], F32)          # vaug pad+ones template
        nc.vector.memset(zo[:, 0:24], 0.0)
        nc.vector.memset(zo[:, 24:25], 1.0)
        eps_t = sbc.tile([128, 1], F32)
        nc.vector.memset(eps_t[:], EPS)
        gqk = sbc.tile([HD, 1], F32)
        nc.sync.dma_start(gqk[:], gqk_d[:])
        bp_b = sbc.tile([128, H], F32)
        nc.sync.dma_start(bp_b[:], bp_d[:])

        def phase_a_tile(b, xTv, m, split=False):
            xc = sbt.tile([128, H], BF16, tag="xc", name=f"xc_{b}_{m}")
            if split:
                # column-sliced load so the first transposes start early
                for g in range(3):
                    nc.gpsimd.dma_start(
                        xc[:, 384 * g:384 * (g + 1)],
                        x_d[b, 128 * m:128 * (m + 1), 384 * g:384 * (g + 1)])
            else:
                nc.gpsimd.dma_start(xc[:], x_d[b, 128 * m:128 * (m + 1), :])
            for g in range(3):  # 3 k-blocks per psum group
                pst = ps_s.tile([128, 1024], BF16, tag="pss",
                                name=f"psx_{b}_{m}_{g}")
                for kk in range(3):
                    kb = 3 * g + kk
                    nc.tensor.transpose(pst[:, 128 * kk:128 * (kk + 1)],
                                        xc[:, 128 * kb:128 * (kb + 1)],
                                        id16[:])
                dst = xTv[:, 3 * g:3 * g + 3, 128 * m:128 * (m + 1)]
                nc.vector.tensor_copy(dst, pst[:, 0:384].rearrange(
                    "p (kk t) -> p kk t", t=128))

        def load_wch(b, tens, coff, chw):
            c0 = tens * H + coff
            wch = sbw.tile([128, 9 * 432], BF16, tag="w",
                           name=f"w{b}_{tens}_{coff}")
            nc.sync.dma_start(
                wch[:].rearrange("p (kb c) -> p kb c", c=432)[:, :, 0:chw],
                wqkv_d[:, c0:c0 + chw].rearrange("(kb p) c -> p kb c", p=128))
            return wch

        next_xTv = None
        for b in range(n_batch):
            # ---------------- phase A: load x, transpose to xT ----------------
            wch0 = None
            if next_xTv is None:
                # prefetch the first weight chunk ahead of the x loads
                wch0 = load_wch(b, 0, CHUNKS[0][0], CHUNKS[0][1])
                xT = sbx.tile([128, 9 * S], BF16, tag="xT", name=f"xT_{b}")
                xTv = xT[:].rearrange("p (kb t) -> p kb t", t=S)
                for m in range(TT):
                    phase_a_tile(b, xTv, m, split=(m == 0))
            else:
                xTv = next_xTv
            next_xTv = None

            # ---------------- phase B: QKV projection ----------------
            q_sb = [sbqk.tile([128, H], BF16, tag=f"q{m}", name=f"q{m}_{b}") for m in range(TT)]
            k_sb = [sbqk.tile([128, H], BF16, tag=f"k{m}", name=f"k{m}_{b}") for m in range(TT)]
            vaug = [sbv.tile([128, 97 * NH], BF16, tag=f"v{m}", name=f"v{m}_{b}") for m in range(TT)]
            stats = sbs.tile([128, 2 * NH * TT], F32, tag="stats",
                             name=f"stats_{b}")
            for m in range(TT):
                nc.vector.tensor_copy(
                    vaug[m][:].rearrange("p (h c) -> p h c", c=97)[:, :, 72:97],
                    zo[:].unsqueeze(1).broadcast_to([128, NH, 25]))
            for tens in range(3):  # 0=q, 1=k, 2=v
                for ci, (coff, chw) in enumerate(CHUNKS):
                    c0 = tens * H + coff
                    nhh = chw // HD
                    h0 = coff // HD
                    if tens == 0 and ci == 0 and wch0 is not None:
                        wch = wch0
                    else:
                        wch = load_wch(b, tens, coff, chw)
                    wv = wch[:].rearrange("p (kb c) -> p kb c", c=432)
                    for m in range(TT):
                        psum = ps_s.tile([128, 512], F32, tag="pss")
                        pr = psum[:, 0:chw]
                        for kb in range(9):
                            nc.tensor.matmul(pr, xTv[:, kb, 128 * m:128 * (m + 1)],
                                             wv[:, kb, 0:chw], start=(kb == 0),
                                             stop=(kb == 8))
                        if tens == 2:  # v -> vaug strided (+bias)
                            dst = vaug[m][:].rearrange("p (h c) -> p h c", c=97)[
                                :, h0:h0 + nhh, 0:72]
                            nc.vector.tensor_tensor(
                                out=dst, in0=pr.rearrange("p (h c) -> p h c", c=HD),
                                in1=bias_b[:, c0:c0 + chw].rearrange(
                                    "p (h c) -> p h c", c=HD), op=ADD)
                        else:
                            dsttile = q_sb[m] if tens == 0 else k_sb[m]
                            nc.vector.tensor_tensor(
                                out=dsttile[:, coff:coff + chw], in0=pr,
                                in1=bias_b[:, c0:c0 + chw], op=ADD)
                            qsq = sbt.tile([128, 432], F32, tag="qsq")
                            nc.scalar.activation(
                                qsq[:, 0:chw], dsttile[:, coff:coff + chw],
                                AF.Square)
                            so = 2 * NH * m + NH * tens + h0
                            nc.vector.reduce_sum(
                                stats[:, so:so + nhh],
                                qsq[:, 0:chw].rearrange("p (h c) -> p h c", c=HD),
                                axis=mybir.AxisListType.X)
            # rinv (batched: one sqrt + one reciprocal per batch) + apply
            rms = sbs.tile([128, 2 * NH * TT], F32, tag="rms", name=f"rms_{b}")
            nc.scalar.activation(rms[:], stats[:], AF.Sqrt,
                                 scale=1.0 / HD, bias=eps_t[:])
            nc.vector.reciprocal(rms[:], rms[:])
            for m in range(TT):
                for tens in range(2):
                    dsttile = q_sb[m] if tens == 0 else k_sb[m]
                    so = 2 * NH * m + NH * tens
                    rb3 = rms[:, so:so + NH].unsqueeze(2) \
                        .broadcast_to([128, NH, HD])
                    dv = dsttile[:].rearrange("p (h c) -> p h c", c=HD)
                    nc.vector.tensor_tensor(out=dv, in0=dv, in1=rb3, op=MUL)

            # ---------------- phase C: attention per head ----------------
            attn = [sba.tile([128, S], BF16, tag=f"a{t}", name=f"a{t}_{b}") for t in range(KT_O)]

            def build_qkT(h, tens):
                src = q_sb if tens == 0 else k_sb
                dst = sbqt.tile([HD, S], BF16, tag=("qT" if tens == 0 else "kT"),
                                name=f"{'qk'[tens]}T_{b}_{h}")
                for g in range(2):  # 4 tok-tiles per psum group
                    pst = ps_s.tile([128, 1024], BF16, tag="pss",
                                    name=f"pst_{b}_{h}_{tens}_{g}")
                    for mm in range(4):
                        m = 4 * g + mm
                        nc.tensor.transpose(
                            pst[0:HD, 128 * mm:128 * (mm + 1)],
                            src[m][:, HD * h:HD * (h + 1)], id16[:])
                    nc.vector.tensor_copy(dst[:, 512 * g:512 * (g + 1)],
                                          pst[0:HD, 0:512])
                if tens == 1:
                    nc.vector.tensor_scalar_mul(dst[:], dst[:], gqk[:])
                return dst

            nxt = (build_qkT(0, 0), build_qkT(0, 1))
            for h in range(NH):
                qT, kT = nxt
                po = ps_pv.tile([128, 1024], F32, tag="pv")
                # software-pipelined: scores for jt+1 issue before PV of jt so
                # the in-order PE stream never stalls on exp(jt)
                def scores(jt):
                    pss = ps_sc.tile([128, 1024], F32, tag="sc",
                                     name=f"pss_{b}_{h}_{jt}")
                    for ih in range(2):
                        nc.tensor.matmul(pss[:, 512 * ih:512 * (ih + 1)],
                                         kT[:, 128 * jt:128 * (jt + 1)],
                                         qT[:, 512 * ih:512 * (ih + 1)],
                                         start=True, stop=True)
                    return pss
                pss_cur = scores(0)
                for jt in range(TT):
                    eT = sbe.tile([128, S], BF16, tag="eT")
                    nc.scalar.activation(eT[:], pss_cur[:], AF.Exp, scale=SCALE)
                    if jt + 1 < TT:
                        pss_cur = scores(jt + 1)
                    # prefetch next head's transposes into exp-wait bubbles
                    if h + 1 < NH and jt == 2:
                        nq = build_qkT(h + 1, 0)
                    elif h + 1 < NH and jt == 5:
                        nxt = (nq, build_qkT(h + 1, 1))
                    elif b + 1 < n_batch and 8 <= h and jt == 7:
                        # overlap next batch's x-load/transpose with attention
                        if h == 8:
                            nxT = sbx.tile([128, 9 * S], BF16, tag="xT",
                                           name=f"xT_{b + 1}")
                            next_xTv = nxT[:].rearrange("p (kb t) -> p kb t",
                                                        t=S)
                        phase_a_tile(b + 1, next_xTv, h - 8)
                    for ih in range(2):
                        nc.tensor.matmul(po[0:97, 512 * ih:512 * (ih + 1)],
                                         vaug[jt][:, 97 * h:97 * h + 97],
                                         eT[:, 512 * ih:512 * (ih + 1)],
                                         start=(jt == 0), stop=(jt == TT - 1))
                # evacuate PV accumulator (frees the psum bank fast), then
                # normalize on-chip: reciprocal of denominator row ->
                # partition_broadcast -> multiply -> pack-DMA into the dense
                # 9-tile attn layout at flat row 72*h.
                posb = sbr.tile([97, S], BF16, tag="posb", name=f"posb_{b}_{h}")
                nc.vector.tensor_copy(posb[:], po[0:97, :])
                rinv = sbr.tile([1, S], F32, tag="rinv", name=f"rinv_{b}_{h}")
                nc.vector.reciprocal(rinv[:], posb[96:97, :])
                rbr = sbr.tile([HD, S], F32, tag="rbr", name=f"rbr_{b}_{h}")
                nc.gpsimd.partition_broadcast(rbr[:], rinv[:])
                stg = sbr.tile([HD, S], BF16, tag="stg", name=f"stg_{b}_{h}")
                nc.gpsimd.tensor_tensor(out=stg[:], in0=posb[0:HD, :],
                                        in1=rbr[:], op=MUL)
                s0 = HD * h
                t0, p0 = s0 // 128, s0 % 128
                len1 = min(HD, 128 - p0)
                nc.sync.dma_start(attn[t0][p0:p0 + len1, :], stg[0:len1, :])
                if len1 < HD:
                    nc.sync.dma_start(attn[t0 + 1][0:HD - len1, :],
                                      stg[len1:HD, :])
                if h == NH - 2:
                    # prefetch out-proj weights for ni=0 ahead of the final
                    # normalize chain so its DMA isn't queued behind it
                    wp0 = sbw.tile([128, KT_O * NP], BF16, tag="w",
                                   name=f"wp{b}_0")
                    nc.sync.dma_start(
                        wp0[:].rearrange("p (kt c) -> p kt c", c=NP),
                        wp_d[:, 0:NP].rearrange("(kt p) c -> p kt c", p=128))

            # ---------------- phase D: out projection ----------------
            for ni in range(H // NP):
                n0 = ni * NP
                if ni == 0:
                    wpch = wp0
                else:
                    wpch = sbw.tile([128, KT_O * NP], BF16, tag="w",
                                    name=f"wp{b}_{ni}")
                    nc.sync.dma_start(
                        wpch[:].rearrange("p (kt c) -> p kt c", c=NP),
                        wp_d[:, n0:n0 + NP].rearrange("(kt p) c -> p kt c",
                                                      p=128))
                wpv = wpch[:].rearrange("p (kt c) -> p kt c", c=NP)
                for m in range(TT):
                    psum = ps_s.tile([128, 512], F32, tag="pss")
                    py = psum[:, 0:NP]
                    for kt in range(KT_O):
                        nc.tensor.matmul(py, attn[kt][:, 128 * m:128 * (m + 1)],
                                         wpv[:, kt, :], start=(kt == 0),
                                         stop=(kt == KT_O - 1))
                    yo = sbt.tile([128, NP], F32, tag="yo")
                    nc.vector.tensor_tensor(out=yo[:], in0=py,
                                            in1=bp_b[:, n0:n0 + NP], op=ADD)
                    nc.sync.dma_start(
                        out_d[b, 128 * m:128 * (m + 1), n0:n0 + NP], yo[:])


_NC_CACHE = {}


def _get_nc(n_batch=B_LOCAL):
    if n_batch not in _NC_CACHE:
        _NC_CACHE[n_batch] = build_nc(n_batch)
    return _NC_CACHE[n_batch]


def prep_inputs(w_qkv, b_qkv, q_gamma, k_gamma, w_proj, b_proj, **_ignored):
    """Host-side layout prep shared by all cores (non-x inputs)."""
    w_qkv = np.asarray(w_qkv, np.float32)
    b_qkv = np.asarray(b_qkv, np.float32)
    q_gamma = np.asarray(q_gamma, np.float32)
    k_gamma = np.asarray(k_gamma, np.float32)
    w_proj = np.asarray(w_proj, np.float32)
    b_proj = np.asarray(b_proj, np.float32)

    wqkvt = np.ascontiguousarray(w_qkv.T).astype(ml_dtypes.bfloat16)  # [H, 3H]
    biasb = np.ascontiguousarray(
        np.broadcast_to(b_qkv, (128, 3 * H))).astype(ml_dtypes.bfloat16)
    gqk = np.ascontiguousarray((q_gamma * k_gamma).reshape(HD, 1))
    wprojt = np.ascontiguousarray(w_proj.T).astype(ml_dtypes.bfloat16)
    bprojb = np.ascontiguousarray(np.broadcast_to(b_proj, (128, H)))
    return {
        "wqkvt": wqkvt, "biasb": biasb, "gqk": gqk,
        "wprojt": wprojt, "bprojb": bprojb,
    }


def run(inputs, trace=False, n_batch=B_LOCAL, n_cores=N_CORES, **run_kwargs):
    """Shard inputs, run SPMD, gather output. Returns (out [B,S,H], results)."""
    x = np.asarray(inputs["x"], np.float32)
    common = prep_inputs(**{k: v for k, v in inputs.items() if k != "x"})
    nc = _get_nc(n_batch)
    in_maps = []
    for c in range(n_cores):
        m = dict(common)
        m["x"] = np.ascontiguousarray(x[c * n_batch:(c + 1) * n_batch])
        in_maps.append(m)
    res = run_bass_kernel_spmd(nc, in_maps, core_ids=list(range(n_cores)),
                               trace=trace, **run_kwargs)
    out = np.concatenate([res.results[c]["out"] for c in range(n_cores)],
                         axis=0)
    return out, res


def kernel(**inputs) -> np.ndarray:
    out, _ = run(inputs)
    return out


# revision 6
# speedup vs baseline: 1.2085x; 1.2085x over previous
"""Trainium2 Bass kernel for nn_Attention: full attention layer
(QKV proj + per-head RMSNorm on q,k + softmax attention + out proj),
data-parallel over batch across 8 NeuronCores (2 batch elems per core).

Per-core dataflow (bf16 compute, f32 PSUM/stats):
  A. x [tok, hid] tiles -> cast-DMA to bf16 -> PE transpose -> xT [hid, tok].
     First weight chunk + first x tile are prefetched in column slices so
     the PE starts within ~3us of kernel entry.
  B. QKV proj in layout [tok, outdim]: lhsT = xT tiles (stationary),
     rhs = w_qkv.T chunks (head-aligned widths 432/432/288); bias added
     during the DVE PSUM evacuation. q,k stored bf16 per token-tile; v
     lands in v_aug tiles [128, 16*97] where each head owns 97 cols:
     72 v | 24 zero | 1 one.
  C. RMSNorm: ACT Square + DVE reduce -> sumsq (one stats tile per batch);
     one ACT Sqrt + one DVE reciprocal per batch; applied in-place via
     broadcast-AP multiply. gamma_q*gamma_k folds into kT per-head scale.
  D. Attention per head, scores TRANSPOSED: sT[j,i] = kT_j.T @ qT so the
     softmax axis is on partitions; exp on ScalarE over [128, 1024] psum
     pairs (no max subtraction: |logit| <= sqrt(72) after RMSNorm);
     PV lhsT = v_aug head slice [128, 97] -> accumulator row 96 is the
     softmax denominator. Normalize: DVE reciprocal of the denominator row
     [1,S], gpsimd partition_broadcast to [72,S], gpsimd multiply into a
     staging tile, then SBUF->SBUF DMA packs head h's 72 rows at flat row
     72h of 9 dense attn k-tiles (no padding). Scores are software-
     pipelined one jt ahead; next head's PE transposes + next batch's
     x-phase interleave into the loop to keep the PE warm.
  E. Out proj over the 9 packed k-tiles of w_proj.T (bf16) + f32 bias.
"""
import sys
import numpy as np

sys.path.insert(0, "/opt/trn_rl_repo")

import concourse.bass as bass  # noqa: E402,F401
import concourse.tile as tile  # noqa: E402
import concourse.mybir as mybir  # noqa: E402
from concourse import bacc  # noqa: E402
from concourse.bass_utils import run_bass_kernel_spmd  # noqa: E402
from concourse.masks import make_identity  # noqa: E402
import ml_dtypes  # noqa: E402

F32 = mybir.dt.float32
F32R = mybir.dt.float32r
BF16 = mybir.dt.bfloat16
AF = mybir.ActivationFunctionType
MUL = mybir.AluOpType.mult
ADD = mybir.AluOpType.add

B, S, H = 16, 1024, 1152
NH, HD = 16, 72
B_LOCAL = 2
N_CORES = 8
TT = S // 128             # 8 token tiles per batch
CHUNKS = [(0, 432), (432, 432), (864, 288)]   # head-aligned proj chunks
KT_O = H // 128           # 9 packed K-tiles for out proj
NP = 384                  # out-proj N chunk
SCALE = 1.0 / float(np.sqrt(HD))
EPS = float(np.finfo(np.float32).eps)


def build_nc(n_batch=B_LOCAL):
    nc = bacc.Bacc("TRN2", target_bir_lowering=False, debug=False,
                   num_devices=N_CORES)
    x_d = nc.dram_tensor("x", [n_batch, S, H], F32, kind="ExternalInput").ap()
    wqkv_d = nc.dram_tensor("wqkvt", [H, 3 * H], BF16, kind="ExternalInput").ap()
    bias_d = nc.dram_tensor("biasb", [128, 3 * H], BF16, kind="ExternalInput").ap()
    gqk_d = nc.dram_tensor("gqk", [HD, 1], F32, kind="ExternalInput").ap()
    wp_d = nc.dram_tensor("wprojt", [H, H], BF16, kind="ExternalInput").ap()
    bp_d = nc.dram_tensor("bprojb", [128, H], F32, kind="ExternalInput").ap()
    out_d = nc.dram_tensor("out", [n_batch, S, H], F32, kind="ExternalOutput").ap()

    with tile.TileContext(nc) as tc:
        _build(nc, tc, n_batch, x_d, wqkv_d, bias_d, gqk_d, wp_d, bp_d, out_d)
    nc.compile()
    return nc


def _build(nc, tc, n_batch, x_d, wqkv_d, bias_d, gqk_d, wp_d, bp_d, out_d):
    import contextlib
    ctx = contextlib.ExitStack()
    with ctx:
        sbc = ctx.enter_context(tc.tile_pool(name="const", bufs=1))
        sbx = ctx.enter_context(tc.tile_pool(name="sbx", bufs=1))
        sbqk = ctx.enter_context(tc.tile_pool(name="sbqk", bufs=1))
        sbv = ctx.enter_context(tc.tile_pool(name="sbv", bufs=1))
        sba = ctx.enter_context(tc.tile_pool(name="sba", bufs=1))
        sbw = ctx.enter_context(tc.tile_pool(name="sbw", bufs=2))
        sbt = ctx.enter_context(tc.tile_pool(name="sbt", bufs=2))
        sbqt = ctx.enter_context(tc.tile_pool(name="sbqt", bufs=2))
        sbs = ctx.enter_context(tc.tile_pool(name="sbs", bufs=1))
        sbr = ctx.enter_context(tc.tile_pool(name="sbr", bufs=3))
        sbe = ctx.enter_context(tc.tile_pool(name="sbe", bufs=3))
        ps_s = ctx.enter_context(tc.tile_pool(name="pss", bufs=2, space="PSUM"))
        ps_sc = ctx.enter_context(tc.tile_pool(name="pssc", bufs=2, space="PSUM"))
        ps_pv = ctx.enter_context(tc.tile_pool(name="pspv", bufs=1, space="PSUM"))

        # constants
        id32 = sbc.tile([128, 128], F32)
        make_identity(nc, id32[:])
        id16 = sbc.tile([128, 128], BF16)
        nc.vector.tensor_copy(id16[:], id32[:])
        bias_b = sbc.tile([128, 3 * H], BF16)
        nc.sync.dma_start(bias_b[:], bias_d[:])
        zo = sbc.tile([128, 25], F32)          # vaug pad+ones template
        nc.vector.memset(zo[:, 0:24], 0.0)
        nc.vector.memset(zo[:, 24:25], 1.0)
        eps_t = sbc.tile([128, 1], F32)
        nc.vector.memset(eps_t[:], EPS)
        gqk = sbc.tile([HD, 1], F32)
        nc.sync.dma_start(gqk[:], gqk_d[:])
        bp_b = sbc.tile([128, H], F32)
        nc.sync.dma_start(bp_b[:], bp_d[:])

        def phase_a_tile(b, xTv, m, split=False):
            xc = sbt.tile([128, H], BF16, tag="xc", name=f"xc_{b}_{m}")
            if split:
                # column-sliced load so the first transposes start early
                for g in range(3):
                    nc.gpsimd.dma_start(
                        xc[:, 384 * g:384 * (g + 1)],
                        x_d[b, 128 * m:128 * (m + 1), 384 * g:384 * (g + 1)])
            else:
                nc.gpsimd.dma_start(xc[:], x_d[b, 128 * m:128 * (m + 1), :])
            for g in range(3):  # 3 k-blocks per psum group
                pst = ps_s.tile([128, 1024], BF16, tag="pss",
                                name=f"psx_{b}_{m}_{g}")
                for kk in range(3):
                    kb = 3 * g + kk
                    nc.tensor.transpose(pst[:, 128 * kk:128 * (kk + 1)],
                                        xc[:, 128 * kb:128 * (kb + 1)],
                                        id16[:])
                dst = xTv[:, 3 * g:3 * g + 3, 128 * m:128 * (m + 1)]
                nc.vector.tensor_copy(dst, pst[:, 0:384].rearrange(
                    "p (kk t) -> p kk t", t=128))

        def load_wch(b, tens, coff, chw):
            c0 = tens * H + coff
            wch = sbw.tile([128, 9 * 432], BF16, tag="w",
                           name=f"w{b}_{tens}_{coff}")
            nc.sync.dma_start(
                wch[:].rearrange("p (kb c) -> p kb c", c=432)[:, :, 0:chw],
                wqkv_d[:, c0:c0 + chw].rearrange("(kb p) c -> p kb c", p=128))
            return wch

        next_xTv = None
        for b in range(n_batch):
            # ---------------- phase A: load x, transpose to xT ----------------
            wch0 = None
            if next_xTv is None:
                # prefetch the first weight chunk ahead of the x loads
                wch0 = load_wch(b, 0, CHUNKS[0][0], CHUNKS[0][1])
                xT = sbx.tile([128, 9 * S], BF16, tag="xT", name=f"xT_{b}")
                xTv = xT[:].rearrange("p (kb t) -> p kb t", t=S)
                for m in range(TT):
                    phase_a_tile(b, xTv, m, split=(m == 0))
            else:
                xTv = next_xTv
            next_xTv = None

            # ---------------- phase B: QKV projection ----------------
            q_sb = [sbqk.tile([128, H], BF16, tag=f"q{m}", name=f"q{m}_{b}") for m in range(TT)]
            k_sb = [sbqk.tile([128, H], BF16, tag=f"k{m}", name=f"k{m}_{b}") for m in range(TT)]
            vaug = [sbv.tile([128, 97 * NH], BF16, tag=f"v{m}", name=f"v{m}_{b}") for m in range(TT)]
            stats = sbs.tile([128, 2 * NH * TT], F32, tag="stats",
                             name=f"stats_{b}")
            for m in range(TT):
                nc.vector.tensor_copy(
                    vaug[m][:].rearrange("p (h c) -> p h c", c=97)[:, :, 72:97],
                    zo[:].unsqueeze(1).broadcast_to([128, NH, 25]))
            for tens in range(3):  # 0=q, 1=k, 2=v
                for ci, (coff, chw) in enumerate(CHUNKS):
                    c0 = tens * H + coff
                    nhh = chw // HD
                    h0 = coff // HD
                    if tens == 0 and ci == 0 and wch0 is not None:
                        wch = wch0
                    else:
                        wch = load_wch(b, tens, coff, chw)
                    wv = wch[:].rearrange("p (kb c) -> p kb c", c=432)
                    for m in range(TT):
                        psum = ps_s.tile([128, 512], F32, tag="pss")
                        pr = psum[:, 0:chw]
                        for kb in range(9):
                            nc.tensor.matmul(pr, xTv[:, kb, 128 * m:128 * (m + 1)],
                                             wv[:, kb, 0:chw], start=(kb == 0),
                                             stop=(kb == 8))
                        if tens == 2:  # v -> vaug strided (+bias)
                            dst = vaug[m][:].rearrange("p (h c) -> p h c", c=97)[
                                :, h0:h0 + nhh, 0:72]
                            nc.vector.tensor_tensor(
                                out=dst, in0=pr.rearrange("p (h c) -> p h c", c=HD),
                                in1=bias_b[:, c0:c0 + chw].rearrange(
                                    "p (h c) -> p h c", c=HD), op=ADD)
                        else:
                            dsttile = q_sb[m] if tens == 0 else k_sb[m]
                            nc.vector.tensor_tensor(
                                out=dsttile[:, coff:coff + chw], in0=pr,
                                in1=bias_b[:, c0:c0 + chw], op=ADD)
                            qsq = sbt.tile([128, 432], F32, tag="qsq")
                            nc.scalar.activation(
                                qsq[:, 0:chw], dsttile[:, coff:coff + chw],
                                AF.Square)
                            so = 2 * NH * m + NH * tens + h0
                            nc.vector.reduce_sum(
                                stats[:, so:so + nhh],
                                qsq[:, 0:chw].rearrange("p (h c) -> p h c", c=HD),
                                axis=mybir.AxisListType.X)
            # rinv (batched: one sqrt + one reciprocal per batch) + apply
            rms = sbs.tile([128, 2 * NH * TT], F32, tag="rms", name=f"rms_{b}")
            nc.scalar.activation(rms[:], stats[:], AF.Sqrt,
                                 scale=1.0 / HD, bias=eps_t[:])
            nc.vector.reciprocal(rms[:], rms[:])
            for m in range(TT):
                for tens in range(2):
                    dsttile = q_sb[m] if tens == 0 else k_sb[m]
                    so = 2 * NH * m + NH * tens
                    rb3 = rms[:, so:so + NH].unsqueeze(2) \
                        .broadcast_to([128, NH, HD])
                    dv = dsttile[:].rearrange("p (h c) -> p h c", c=HD)
                    nc.vector.tensor_tensor(out=dv, in0=dv, in1=rb3, op=MUL)

            # ---------------- phase C: attention per head ----------------
            attn = [sba.tile([128, S], BF16, tag=f"a{t}", name=f"a{t}_{b}") for t in range(KT_O)]

            def build_qkT(h, tens):
                src = q_sb if tens == 0 else k_sb
                dst = sbqt.tile([HD, S], BF16, tag=("qT" if tens == 0 else "kT"),
                                name=f"{'qk'[tens]}T_{b}_{h}")
                for g in range(2):  # 4 tok-tiles per psum group
                    pst = ps_s.tile([128, 1024], BF16, tag="pss",
                                    name=f"pst_{b}_{h}_{tens}_{g}")
                    for mm in range(4):
                        m = 4 * g + mm
                        nc.tensor.transpose(
                            pst[0:HD, 128 * mm:128 * (mm + 1)],
                            src[m][:, HD * h:HD * (h + 1)], id16[:])
                    nc.vector.tensor_copy(dst[:, 512 * g:512 * (g + 1)],
                                          pst[0:HD, 0:512])
                if tens == 1:
                    nc.vector.tensor_scalar_mul(dst[:], dst[:], gqk[:])
                return dst

            nxt = (build_qkT(0, 0), build_qkT(0, 1))
            for h in range(NH):
                qT, kT = nxt
                po = ps_pv.tile([128, 1024], F32, tag="pv")
                # software-pipelined: scores for jt+1 issue before PV of jt so
                # the in-order PE stream never stalls on exp(jt)
                def scores(jt):
                    pss = ps_sc.tile([128, 1024], F32, tag="sc",
                                     name=f"pss_{b}_{h}_{jt}")
                    for ih in range(2):
                        nc.tensor.matmul(pss[:, 512 * ih:512 * (ih + 1)],
                                         kT[:, 128 * jt:128 * (jt + 1)],
                                         qT[:, 512 * ih:512 * (ih + 1)],
                                         start=True, stop=True)
                    return pss
                pss_cur = scores(0)
                for jt in range(TT):
                    eT = sbe.tile([128, S], BF16, tag="eT")
                    nc.scalar.activation(eT[:], pss_cur[:], AF.Exp, scale=SCALE)
                    if jt + 1 < TT:
                        pss_cur = scores(jt + 1)
                    # prefetch next head's transposes into exp-wait bubbles
                    if h + 1 < NH and jt == 2:
                        nq = build_qkT(h + 1, 0)
                    elif h + 1 < NH and jt == 5:
                        nxt = (nq, build_qkT(h + 1, 1))
                    elif b + 1 < n_batch and 8 <= h and jt == 7:
                        # overlap next batch's x-load/transpose with attention
                        if h == 8:
                            nxT = sbx.tile([128, 9 * S], BF16, tag="xT",
                                           name=f"xT_{b + 1}")
                            next_xTv = nxT[:].rearrange("p (kb t) -> p kb t",
                                                        t=S)
                        phase_a_tile(b + 1, next_xTv, h - 8)
                    for ih in range(2):
                        nc.tensor.matmul(po[0:97, 512 * ih:512 * (ih + 1)],
                                         vaug[jt][:, 97 * h:97 * h + 97],
                                         eT[:, 512 * ih:512 * (ih + 1)],
                                         start=(jt == 0), stop=(jt == TT - 1))
                # evacuate PV accumulator (frees the psum bank fast), then
                # normalize on-chip: reciprocal of denominator row ->
                # partition_broadcast -> multiply -> pack-DMA into the dense
                # 9-tile attn layout at flat row 72*h.
                posb = sbr.tile([HD, S], BF16, tag="posb", name=f"posb_{b}_{h}")
                nc.vector.tensor_copy(posb[:], po[0:HD, :])
                rinv = sbr.tile([1, S], F32, tag="rinv", name=f"rinv_{b}_{h}")
                dnf = sbr.tile([1, S], F32, tag="dnf", name=f"dnf_{b}_{h}")
                nc.vector.tensor_copy(dnf[:], po[96:97, :])
                nc.vector.reciprocal_approx_fast(rinv[:], dnf[:])
                rbr = sbr.tile([HD, S], F32, tag="rbr", name=f"rbr_{b}_{h}")
                nc.gpsimd.partition_broadcast(rbr[:], rinv[:])
                stg = sbr.tile([HD, S], BF16, tag="stg", name=f"stg_{b}_{h}")
                nc.gpsimd.tensor_tensor(out=stg[:], in0=posb[:],
                                        in1=rbr[:], op=MUL)
                s0 = HD * h
                t0, p0 = s0 // 128, s0 % 128
                len1 = min(HD, 128 - p0)
                nc.sync.dma_start(attn[t0][p0:p0 + len1, :], stg[0:len1, :])
                if len1 < HD:
                    nc.sync.dma_start(attn[t0 + 1][0:HD - len1, :],
                                      stg[len1:HD, :])
                if h == NH - 2:
                    # prefetch out-proj weights for ni=0 ahead of the final
                    # normalize chain so its DMA isn't queued behind it
                    wp0 = sbw.tile([128, KT_O * NP], BF16, tag="w",
                                   name=f"wp{b}_0")
                    nc.sync.dma_start(
                        wp0[:].rearrange("p (kt c) -> p kt c", c=NP),
                        wp_d[:, 0:NP].rearrange("(kt p) c -> p kt c", p=128))

            # ---------------- phase D: out projection ----------------
            for ni in range(H // NP):
                n0 = ni * NP
                if ni == 0:
                    wpch = wp0
                else:
                    wpch = sbw.tile([128, KT_O * NP], BF16, tag="w",
                                    name=f"wp{b}_{ni}")
                    nc.sync.dma_start(
                        wpch[:].rearrange("p (kt c) -> p kt c", c=NP),
                        wp_d[:, n0:n0 + NP].rearrange("(kt p) c -> p kt c",
                                                      p=128))
                wpv = wpch[:].rearrange("p (kt c) -> p kt c", c=NP)
                for m in range(TT):
                    psum = ps_s.tile([128, 512], F32, tag="pss")
                    py = psum[:, 0:NP]
                    for kt in range(KT_O):
                        nc.tensor.matmul(py, attn[kt][:, 128 * m:128 * (m + 1)],
                                         wpv[:, kt, :], start=(kt == 0),
                                         stop=(kt == KT_O - 1))
                    yo = sbt.tile([128, NP], F32, tag="yo")
                    nc.vector.tensor_tensor(out=yo[:], in0=py,
                                            in1=bp_b[:, n0:n0 + NP], op=ADD)
                    nc.sync.dma_start(
                        out_d[b, 128 * m:128 * (m + 1), n0:n0 + NP], yo[:])


_NC_CACHE = {}


def _get_nc(n_batch=B_LOCAL):
    if n_batch not in _NC_CACHE:
        _NC_CACHE[n_batch] = build_nc(n_batch)
    return _NC_CACHE[n_batch]


def prep_inputs(w_qkv, b_qkv, q_gamma, k_gamma, w_proj, b_proj, **_ignored):
    """Host-side layout prep shared by all cores (non-x inputs)."""
    w_qkv = np.asarray(w_qkv, np.float32)
    b_qkv = np.asarray(b_qkv, np.float32)
    q_gamma = np.asarray(q_gamma, np.float32)
    k_gamma = np.asarray(k_gamma, np.float32)
    w_proj = np.asarray(w_proj, np.float32)
    b_proj = np.asarray(b_proj, np.float32)

    wqkvt = np.ascontiguousarray(w_qkv.T).astype(ml_dtypes.bfloat16)  # [H, 3H]
    biasb = np.ascontiguousarray(
        np.broadcast_to(b_qkv, (128, 3 * H))).astype(ml_dtypes.bfloat16)
    gqk = np.ascontiguousarray((q_gamma * k_gamma).reshape(HD, 1))
    wprojt = np.ascontiguousarray(w_proj.T).astype(ml_dtypes.bfloat16)
    bprojb = np.ascontiguousarray(np.broadcast_to(b_proj, (128, H)))
    return {
        "wqkvt": wqkvt, "biasb": biasb, "gqk": gqk,
        "wprojt": wprojt, "bprojb": bprojb,
    }


def run(inputs, trace=False, n_batch=B_LOCAL, n_cores=N_CORES, **run_kwargs):
    """Shard inputs, run SPMD, gather output. Returns (out [B,S,H], results)."""
    x = np.asarray(inputs["x"], np.float32)
    common = prep_inputs(**{k: v for k, v in inputs.items() if k != "x"})
    nc = _get_nc(n_batch)
    in_maps = []
    for c in range(n_cores):
        m = dict(common)
        m["x"] = np.ascontiguousarray(x[c * n_batch:(c + 1) * n_batch])
        in_maps.append(m)
    res = run_bass_kernel_spmd(nc, in_maps, core_ids=list(range(n_cores)),
                               trace=trace, **run_kwargs)
    out = np.concatenate([res.results[c]["out"] for c in range(n_cores)],
                         axis=0)
    return out, res


def kernel(**inputs) -> np.ndarray:
    out, _ = run(inputs)
    return out
